# revision 10
# baseline (speedup 1.0000x reference)
"""Bass/Trainium2 kernel for nn_Epdiff: Hermitian-truncated EPDiff smoothing
filters.

reference:
    cc(g) = -2*cos(2*pi*g) + 2
    coeff_sum[i,j,k] = cc(gx)[i] + cc(gy)[j] + cc(gz)[k]      (gx,gy 2m-band, gz m)
    val = (3*coeff_sum + 1)**6                                [2m, 2m, m]
    res_smooth = 1/val, res_sharp = val, broadcast to [B, 1, 2m, 2m, m]

Strategy (8 cores, batch-sharded): every core computes the full [128, 8192]
filter plane (partition axis = x, free axis = y*64+z) and writes its 4-batch
shard of both outputs.  Host only precomputes the 320 cosine coefficients;
all O(MB) work happens on-device:
  - DMA partition-broadcast of byz = cc(gy) (+) cc(gz)  into SBUF chunks
  - ACT:  v2 = Square(3*byz + bias_x)   with bias_x = 3*cc(gx)+1  per-partition
  - DVE:  v4 = v2*v2 ; v6 = v4*v2       (matches XLA's x**6 repeated squaring)
  - DVE:  rc = reciprocal(v6)
  - DMA:  v6 -> sharp[b], rc -> smooth[b]  for each local batch b
"""

import os
import numpy as np

# ---- problem constants (hardcoded per spec) ----
MODE = 64
TWO_M = 2 * MODE            # 128 partitions
FREE = TWO_M * MODE         # 8192 = y*z free dim
BATCH = 32
N_CORES = 8
B_LOC = BATCH // N_CORES    # 4
NCHUNK = 4
CH = FREE // NCHUNK         # 2048 -> 1 MiB per [128, CH] f32 tile
ALPHA = 3.0
GAMMA = 1.0

_NC = None                  # compiled Bass module, cached per process
LAST_RESULTS = None         # BassKernelResults of the most recent run (for test.py)


def _ensure_path():
    try:
        import concourse.bass  # noqa: F401
        return
    except ImportError:
        pass
    import sys
    for p in ("/opt/trn_rl_repo", "/root/.axon_site/_ro/trn_rl_repo"):
        if os.path.isdir(p) and p not in sys.path:
            sys.path.insert(0, p)


def _legalize_single_wait(nc):
    """This container's walrus build rejects any instruction carrying more
    than one semaphore wait ("Too many sync wait commands"), including the
    Tile-generated kernel-tail Drain.  Split every multi-wait instruction
    into a chain of single-wait Drains on the same engine followed by the
    original instruction with its last wait."""
    from concourse import mybir

    n_new = 0
    for fn in nc.m.functions:
        for bb in fn.blocks:
            insts = bb.instructions
            idx = 0
            while idx < len(insts):
                inst = insts[idx]
                si = inst.sync_info
                if si is not None and len(si.on_wait) > 1:
                    waits = list(si.on_wait)
                    eng = inst.engine
                    for k, w in enumerate(waits[:-1]):
                        d = mybir.InstDrain(name=f"{inst.name}-sw{k}")
                        d.sync_info = mybir.SyncInfo(on_wait=[w], on_update=[])
                        d.engine = eng
                        insts.insert(idx, d)
                        idx += 1
                        n_new += 1
                    inst.sync_info = mybir.SyncInfo(
                        on_wait=[waits[-1]], on_update=list(si.on_update)
                    )
                idx += 1
    return n_new


def _build_nc():
    from concourse import bass, mybir
    import concourse.tile as tile

    f32 = mybir.dt.float32
    nc = bass.Bass()

    byz = nc.dram_tensor("byz", [FREE], f32, kind="ExternalInput")
    biasx = nc.dram_tensor("biasx", [TWO_M], f32, kind="ExternalInput")
    sharp = nc.dram_tensor("sharp", [B_LOC, TWO_M, FREE], f32, kind="ExternalOutput")
    smooth = nc.dram_tensor("smooth", [B_LOC, TWO_M, FREE], f32, kind="ExternalOutput")

    with tile.TileContext(nc) as tc:
        with (
            tc.tile_pool(name="const", bufs=1) as cpool,
            tc.tile_pool(name="work", bufs=2) as wpool,
            tc.tile_pool(name="out", bufs=NCHUNK) as opool,
        ):
            bias_t = cpool.tile([TWO_M, 1], f32)
            nc.gpsimd.dma_start(bias_t[:], biasx[:, None])
            # TRN2 instructions take at most ONE sem wait; touch bias_t on
            # DVE now so the chunk-0 tensor_scalar doesn't need a second
            # wait for it on top of its bt-fill wait.
            bias_obs = cpool.tile([TWO_M, 1], f32)
            nc.vector.tensor_copy(bias_obs[:], bias_t[:])

            for i in range(NCHUNK):
                sl = bass.ts(i, CH)
                # partition-broadcast byz chunk into all 128 rows
                # (own slot per chunk: a fill must not combine a WAR wait on
                # DVE with its DMA-lane wait — one sem wait per instruction)
                bt = opool.tile([TWO_M, CH], f32, tag="bt")
                nc.gpsimd.dma_start(bt[:], byz[None, sl].broadcast_to((TWO_M, CH)))

                # s = 3*byz + (3*cc(gx)+1)  — one DVE tensor_scalar (2x f32 mode)
                s = wpool.tile([TWO_M, CH], f32, tag="s")
                nc.vector.tensor_scalar(
                    s[:], bt[:], ALPHA, bias_t[:, 0:1],
                    mybir.AluOpType.mult, mybir.AluOpType.add,
                )
                # v6 = s^6 via repeated squaring (matches XLA's x**6)
                v2 = wpool.tile([TWO_M, CH], f32, tag="v2")
                nc.vector.tensor_mul(v2[:], s[:], s[:])
                v4 = wpool.tile([TWO_M, CH], f32, tag="v4")
                nc.vector.tensor_mul(v4[:], v2[:], v2[:])
                # v6/rc are read by the output DMAs (scattered across DMA sem
                # lanes) — give them one slot per chunk so slot-reuse never
                # creates multi-sem WAR waits on a DVE instruction.
                v6 = opool.tile([TWO_M, CH], f32, tag="v6")
                nc.vector.tensor_mul(v6[:], v4[:], v2[:])
                rc = opool.tile([TWO_M, CH], f32, tag="rc")
                nc.vector.reciprocal(rc[:], v6[:])

                # one DMA per output per chunk: replicate across the 4 batch
                # slots via a 0-stride middle dim on the SBUF source (keeps
                # total HWDGE DMAs <= the 8 sem lanes -> single-wait insts)
                nc.sync.dma_start(
                    sharp[:, :, sl].rearrange("b p c -> p b c"),
                    v6[:, None, :].broadcast_to((TWO_M, B_LOC, CH)),
                )
                nc.sync.dma_start(
                    smooth[:, :, sl].rearrange("b p c -> p b c"),
                    rc[:, None, :].broadcast_to((TWO_M, B_LOC, CH)),
                )

    _legalize_single_wait(nc)
    return nc


def kernel(gridx, gridy, gridz, mode, batchsize):
    _ensure_path()
    global _NC, LAST_RESULTS
    from concourse.bass_utils import run_bass_kernel_spmd

    m = int(mode)
    bsz = int(batchsize)
    assert m == MODE and bsz == BATCH, (m, bsz)

    gridx = np.asarray(gridx, np.float32)
    gridy = np.asarray(gridy, np.float32)
    gridz = np.asarray(gridz, np.float32)

    def cc(g):
        # f32 throughout, matching the f32 reference
        return (np.float32(-2.0) * np.cos(np.float32(2.0 * np.pi) * g)
                + np.float32(2.0))

    ccx = cc(np.concatenate([gridx[:m], gridx[-m:]]))   # [128]
    ccy = cc(np.concatenate([gridy[:m], gridy[-m:]]))   # [128]
    ccz = cc(gridz[:m])                                 # [64]

    byz = (ccy[:, None] + ccz[None, :]).reshape(-1).astype(np.float32)   # [8192]
    biasx = (np.float32(ALPHA) * ccx + np.float32(GAMMA)).astype(np.float32)  # [128]

    if _NC is None:
        _NC = _build_nc()

    in_maps = [{"byz": byz, "biasx": biasx} for _ in range(N_CORES)]
    res = run_bass_kernel_spmd(_NC, in_maps, core_ids=list(range(N_CORES)))
    LAST_RESULTS = res

    sharp = np.concatenate(
        [r["sharp"].reshape(B_LOC, 1, TWO_M, TWO_M, MODE) for r in res.results], axis=0
    )
    smooth = np.concatenate(
        [r["smooth"].reshape(B_LOC, 1, TWO_M, TWO_M, MODE) for r in res.results], axis=0
    )
    return (smooth, sharp)
